# revision 1
# baseline (speedup 1.0000x reference)
"""Self-contained Trainium2 Bass kernel for the 2-layer GAT problem.

Accepts FULL inputs, shards destination-node ranges across 8 NeuronCores
internally, and returns the FULL [50000, 2] float32 output.
"""
import numpy as np

import concourse.bacc as bacc
import concourse.mybir as mybir
import concourse.tile as tile
from concourse.masks import make_identity

F32 = mybir.dt.float32
BF = mybir.dt.bfloat16
I16 = mybir.dt.int16
NP_BF = mybir.dt.np(BF)

H = 8       # heads
C = 32      # per-head channels
HD = H * C  # 256
FIN = 128
ELEM = 384
ELEM2 = 128
P = 128

FULL_CFG = dict(
    N=50000, NPAD=50176, PER=6272, NBLK=49, HALF=25088, NSUB=10, NCORES=8,
    XCHUNK=1024,
)


def build_nc(cfg):
    NPAD, PER, NBLK, HALF, NSUB = (
        cfg["NPAD"], cfg["PER"], cfg["NBLK"], cfg["HALF"], cfg["NSUB"])
    NCORES = cfg["NCORES"]
    XCHUNK = cfg["XCHUNK"]
    NEH = NSUB * P                # idxs per (block, half) gather
    IDXW = NEH // 16              # idx cols per bucket
    NTILE = NPAD // P             # node tiles in P1
    assert NPAD == NCORES * PER and PER == NBLK * P and NPAD % XCHUNK == 0
    assert HALF % P == 0 and 2 * HALF == NPAD

    nc = bacc.Bacc(None, target_bir_lowering=False, num_devices=NCORES)

    xT_d = nc.dram_tensor("xT", [FIN, NPAD], F32, kind="ExternalInput")
    w1e_d = nc.dram_tensor("w1e", [FIN, 272], F32, kind="ExternalInput")
    w2e_d = nc.dram_tensor("w2e", [P, 8], BF, kind="ExternalInput")
    b1_d = nc.dram_tensor("b1r", [1, HD], F32, kind="ExternalInput")
    b2_d = nc.dram_tensor("b2r", [1, 2], F32, kind="ExternalInput")
    idx_d = nc.dram_tensor("idx16", [P, NBLK * 2 * IDXW], I16, kind="ExternalInput")
    dst_d = nc.dram_tensor("dstf", [P, NBLK * 2 * NSUB], F32, kind="ExternalInput")
    out_d = nc.dram_tensor("out", [PER, 2], F32, kind="ExternalOutput")

    table = nc.dram_tensor("table", [NPAD, ELEM], BF)
    t2loc = nc.dram_tensor("t2loc", [PER, ELEM2], BF)
    table2 = nc.dram_tensor("table2", [NPAD, ELEM2], BF)

    with tile.TileContext(nc) as tc:
        with (
            tc.tile_pool(name="cst", bufs=1) as cst,
            tc.tile_pool(name="xp", bufs=2) as xp,
            tc.tile_pool(name="rowp", bufs=3) as rowp,
            tc.tile_pool(name="gp", bufs=2) as gp,
            tc.tile_pool(name="g2p", bufs=2) as g2p,
            tc.tile_pool(name="wk", bufs=3) as wk,
            tc.tile_pool(name="tailp", bufs=2) as tailp,
            tc.tile_pool(name="ps", bufs=2, space="PSUM") as ps,
        ):
            # ---- constants ----
            ident = cst.tile([P, P], BF)
            make_identity(nc, ident[:])
            iota_i = cst.tile([P, P], I16)
            nc.gpsimd.iota(iota_i[:], pattern=[[1, P]], base=0, channel_multiplier=0)
            iota_bf = cst.tile([P, P], BF)
            nc.vector.tensor_copy(iota_bf[:], iota_i[:])
            onesk = cst.tile([1, P], F32)
            nc.vector.memset(onesk[:], 1.0)

            w1e_sb = cst.tile([FIN, 272], F32)
            nc.sync.dma_start(out=w1e_sb[:], in_=w1e_d[:])
            w2e_sb = cst.tile([P, 2, 4], BF)
            nc.sync.dma_start(out=w2e_sb[:], in_=w2e_d[:].rearrange("p (k n) -> p k n", k=2))
            idx_sb = cst.tile([P, NBLK * 2 * IDXW], I16)
            nc.sync.dma_start(out=idx_sb[:], in_=idx_d[:])
            dst_sb = cst.tile([P, NBLK * 2 * NSUB], F32)
            nc.sync.dma_start(out=dst_sb[:], in_=dst_d[:])

            # bias broadcast rows -> [P, HD], [P, 2]
            b1r = cst.tile([1, HD], F32)
            nc.sync.dma_start(out=b1r[:], in_=b1_d[:])
            b2r = cst.tile([1, 2], F32)
            nc.sync.dma_start(out=b2r[:], in_=b2_d[:])
            bps = ps.tile([P, HD], F32, space="PSUM", tag="accum")
            nc.tensor.matmul(out=bps[:], lhsT=onesk[:], rhs=b1r[:], start=True, stop=True)
            b1bc = cst.tile([P, HD], F32)
            nc.scalar.copy(b1bc[:], bps[:])
            bps2 = ps.tile([P, 2], F32, space="PSUM", tag="accum")
            nc.tensor.matmul(out=bps2[:], lhsT=onesk[:], rhs=b2r[:], start=True, stop=True)
            b2bc = cst.tile([P, 2], F32)
            nc.scalar.copy(b2bc[:], bps2[:])

            # ---- P1: node features -> table (replicated over all nodes) ----
            for ch in range(NPAD // XCHUNK):
                xc = xp.tile([FIN, XCHUNK], F32, tag="xc")
                nc.sync.dma_start(out=xc[:], in_=xT_d[:, ch * XCHUNK:(ch + 1) * XCHUNK])
                for j in range(XCHUNK // P):
                    nt = ch * (XCHUNK // P) + j
                    ph = ps.tile([P, 272], F32, space="PSUM", tag="accum")
                    nc.tensor.matmul(out=ph[:], lhsT=xc[:, j * P:(j + 1) * P],
                                     rhs=w1e_sb[:], start=True, stop=True)
                    row = rowp.tile([P, 280], BF, tag="row")
                    r3 = row[:, 0:264].rearrange("p (h x) -> p h x", h=H)
                    nc.scalar.copy(r3[:, :, 0:C],
                                   ph[:, 0:HD].rearrange("p (h c) -> p h c", h=H))
                    nc.gpsimd.memset(r3[:, :, C:C + 1], 1.0)
                    nc.scalar.copy(row[:, 264:280], ph[:, HD:HD + 16])
                    nc.sync.dma_start(out=table[nt * P:(nt + 1) * P, 0:280], in_=row[:])

            # ---- adst slice for own dst range (pid ladder) ----
            adst_sb = cst.tile([P, NBLK, H], BF)
            pid = nc.sync.partition_id()
            for c in range(NCORES):
                with tc.If(pid == c):
                    nc.sync.dma_start(
                        out=adst_sb[:],
                        in_=table[c * PER:(c + 1) * PER, 272:280]
                            .rearrange("(b p) h -> p b h", p=P))

            adst2_sb = cst.tile([P, NBLK], BF)

            # ---- P2: layer-1 message passing over own dst blocks ----
            for b in range(NBLK):
                pblk = ps.tile([P, 264], F32, space="PSUM", tag="accum")
                for half in (0, 1):
                    bucket = b * 2 + half
                    g = gp.tile([P, NSUB, ELEM], BF, tag="g")
                    nc.gpsimd.dma_gather(
                        out_ap=g[:],
                        in_ap=(table[0:HALF, :] if half == 0 else table[HALF:NPAD, :]),
                        idxs_ap=idx_sb[:, bucket * IDXW:(bucket + 1) * IDXW],
                        num_idxs=NEH, num_idxs_reg=NEH, elem_size=ELEM,
                        single_packet=False)
                    Ss = []
                    aexp = ps.tile([P, NSUB, H], F32, space="PSUM", tag="aexp")
                    for t in range(NSUB):
                        col = bucket * NSUB + t
                        S = wk.tile([P, P], BF, tag=f"S{t}", bufs=2)
                        nc.vector.tensor_scalar(
                            out=S[:], in0=iota_bf[:], scalar1=dst_sb[:, col:col + 1],
                            scalar2=None, op0=mybir.AluOpType.is_equal)
                        Ss.append(S)
                        T_ps = ps.tile([P, P], BF, space="PSUM", tag="tps")
                        nc.tensor.transpose(T_ps[:], S[:], ident[:])
                        T_sb = wk.tile([P, P], BF, tag="T_sb")
                        nc.scalar.copy(T_sb[:], T_ps[:])
                        nc.tensor.matmul(out=aexp[:, t, :], lhsT=T_sb[:],
                                         rhs=adst_sb[:, b, :], start=True, stop=True)
                    logits = wk.tile([P, NSUB, H], F32, tag="logits")
                    nc.vector.tensor_tensor(out=logits[:], in0=g[:, :, 264:272],
                                            in1=aexp[:], op=mybir.AluOpType.add)
                    e1 = wk.tile([P, NSUB, H], F32, tag="e1")
                    nc.scalar.activation(e1[:], logits[:], mybir.ActivationFunctionType.Exp)
                    e2 = wk.tile([P, NSUB, H], F32, tag="e2")
                    nc.scalar.activation(e2[:], logits[:], mybir.ActivationFunctionType.Exp,
                                         scale=0.2)
                    wt = wk.tile([P, NSUB, H], BF, tag="wt")
                    nc.vector.tensor_tensor(out=wt[:], in0=e1[:], in1=e2[:],
                                            op=mybir.AluOpType.max)
                    for t in range(NSUB):
                        msg = wk.tile([P, 264], BF, tag=f"msg{t % 3}")
                        nc.vector.tensor_tensor(
                            out=msg[:].rearrange("p (h x) -> p h x", h=H),
                            in0=g[:, t, 0:264].rearrange("p (h x) -> p h x", h=H),
                            in1=wt[:, t, :, None].to_broadcast([P, H, C + 1]),
                            op=mybir.AluOpType.mult)
                        nc.tensor.matmul(out=pblk[:], lhsT=Ss[t][:], rhs=msg[:],
                                         start=(half == 0 and t == 0),
                                         stop=(half == 1 and t == NSUB - 1))
                # tail: normalize + bias + ELU -> h2 -> t2loc rows
                pb3 = pblk[:].rearrange("p (h x) -> p h x", h=H)
                srec = tailp.tile([P, H], F32, tag="srec")
                nc.vector.tensor_scalar(
                    out=srec[:], in0=pb3[:, :, C:C + 1].rearrange("p h x -> p (h x)"),
                    scalar1=1e-16, scalar2=None, op0=mybir.AluOpType.add)
                rec = tailp.tile([P, H], F32, tag="rec")
                nc.vector.reciprocal(rec[:], srec[:])
                out1 = tailp.tile([P, HD], F32, tag="out1")
                nc.vector.tensor_tensor(
                    out=out1[:].rearrange("p (h c) -> p h c", h=H),
                    in0=pb3[:, :, 0:C],
                    in1=rec[:, :, None].to_broadcast([P, H, C]),
                    op=mybir.AluOpType.mult)
                v = tailp.tile([P, HD], F32, tag="v")
                nc.vector.tensor_tensor(out=v[:], in0=out1[:], in1=b1bc[:],
                                        op=mybir.AluOpType.add)
                ev = tailp.tile([P, HD], F32, tag="ev")
                nc.scalar.activation(ev[:], v[:], mybir.ActivationFunctionType.Exp)
                em = tailp.tile([P, HD], F32, tag="em")
                nc.vector.tensor_scalar(out=em[:], in0=ev[:], scalar1=1.0, scalar2=0.0,
                                        op0=mybir.AluOpType.subtract,
                                        op1=mybir.AluOpType.min)
                pp = tailp.tile([P, HD], F32, tag="pp")
                nc.scalar.activation(pp[:], v[:], mybir.ActivationFunctionType.Relu)
                elu = tailp.tile([P, HD], BF, tag="elu")
                nc.vector.tensor_tensor(out=elu[:], in0=em[:], in1=pp[:],
                                        op=mybir.AluOpType.add)
                eT_sb = tailp.tile([P, 2, P], BF, tag="eT_sb")
                ph2 = ps.tile([P, 4], F32, space="PSUM", tag="tail")
                for k in range(2):
                    eT_ps = ps.tile([P, P], BF, space="PSUM", tag="tps")
                    nc.tensor.transpose(eT_ps[:], elu[:, k * P:(k + 1) * P], ident[:])
                    nc.scalar.copy(eT_sb[:, k, :], eT_ps[:])
                for k in range(2):
                    nc.tensor.matmul(out=ph2[:], lhsT=eT_sb[:, k, :], rhs=w2e_sb[:, k, :],
                                     start=(k == 0), stop=(k == 1))
                t2row = tailp.tile([P, ELEM2], BF, tag="t2row")
                nc.scalar.copy(t2row[:, 0:2], ph2[:, 0:2])
                nc.gpsimd.memset(t2row[:, 2:3], 1.0)
                nc.scalar.copy(t2row[:, 3:5], ph2[:, 2:4])
                nc.gpsimd.memset(t2row[:, 5:ELEM2], 0.0)
                nc.sync.dma_start(out=t2loc[b * P:(b + 1) * P, :], in_=t2row[:])
                nc.scalar.copy(adst2_sb[:, b:b + 1], ph2[:, 3:4])

            # ---- AllGather layer-2 node table ----
            nc.gpsimd.collective_compute(
                "AllGather", mybir.AluOpType.bypass,
                replica_groups=[list(range(NCORES))],
                ins=[t2loc[:]], outs=[table2[:]])

            # ---- P3: layer-2 message passing ----
            for b in range(NBLK):
                p2s = ps.tile([P, 3], F32, space="PSUM", tag="accum")
                for half in (0, 1):
                    bucket = b * 2 + half
                    g2 = g2p.tile([P, NSUB, ELEM2], BF, tag="g2")
                    nc.gpsimd.dma_gather(
                        out_ap=g2[:],
                        in_ap=(table2[0:HALF, :] if half == 0 else table2[HALF:NPAD, :]),
                        idxs_ap=idx_sb[:, bucket * IDXW:(bucket + 1) * IDXW],
                        num_idxs=NEH, num_idxs_reg=NEH, elem_size=ELEM2,
                        single_packet=False)
                    S2s = []
                    a2e = ps.tile([P, NSUB], F32, space="PSUM", tag="aexp")
                    for t in range(NSUB):
                        col = bucket * NSUB + t
                        S2 = wk.tile([P, P], BF, tag=f"S{t}", bufs=2)
                        nc.vector.tensor_scalar(
                            out=S2[:], in0=iota_bf[:], scalar1=dst_sb[:, col:col + 1],
                            scalar2=None, op0=mybir.AluOpType.is_equal)
                        S2s.append(S2)
                        T2_ps = ps.tile([P, P], BF, space="PSUM", tag="tps")
                        nc.tensor.transpose(T2_ps[:], S2[:], ident[:])
                        T2_sb = wk.tile([P, P], BF, tag="T_sb")
                        nc.vector.tensor_copy(T2_sb[:], T2_ps[:])
                        nc.tensor.matmul(out=a2e[:, t:t + 1], lhsT=T2_sb[:],
                                         rhs=adst2_sb[:, b:b + 1], start=True, stop=True)
                    lg2 = wk.tile([P, NSUB], F32, tag="logits2")
                    nc.vector.tensor_tensor(
                        out=lg2[:], in0=g2[:, :, 3:4].rearrange("p t x -> p (t x)"),
                        in1=a2e[:], op=mybir.AluOpType.add)
                    f1 = wk.tile([P, NSUB], F32, tag="f1")
                    nc.scalar.activation(f1[:], lg2[:], mybir.ActivationFunctionType.Exp)
                    f2 = wk.tile([P, NSUB], F32, tag="f2")
                    nc.scalar.activation(f2[:], lg2[:], mybir.ActivationFunctionType.Exp,
                                         scale=0.2)
                    w2t = wk.tile([P, NSUB], BF, tag="w2t")
                    nc.vector.tensor_tensor(out=w2t[:], in0=f1[:], in1=f2[:],
                                            op=mybir.AluOpType.max)
                    for t in range(NSUB):
                        msg2 = wk.tile([P, 3], BF, tag=f"msg2{t % 3}")
                        nc.vector.tensor_tensor(
                            out=msg2[:], in0=g2[:, t, 0:3],
                            in1=w2t[:, t:t + 1].to_broadcast([P, 3]),
                            op=mybir.AluOpType.mult)
                        nc.tensor.matmul(out=p2s[:], lhsT=S2s[t][:], rhs=msg2[:],
                                         start=(half == 0 and t == 0),
                                         stop=(half == 1 and t == NSUB - 1))
                s2r = tailp.tile([P, 1], F32, tag="s2r")
                nc.vector.tensor_scalar(out=s2r[:], in0=p2s[:, 2:3], scalar1=1e-16,
                                        scalar2=None, op0=mybir.AluOpType.add)
                rec2 = tailp.tile([P, 1], F32, tag="rec2")
                nc.vector.reciprocal(rec2[:], s2r[:])
                o2 = tailp.tile([P, 2], F32, tag="o2")
                nc.vector.tensor_tensor(out=o2[:], in0=p2s[:, 0:2],
                                        in1=rec2[:].to_broadcast([P, 2]),
                                        op=mybir.AluOpType.mult)
                o2b = tailp.tile([P, 2], F32, tag="o2b")
                nc.vector.tensor_tensor(out=o2b[:], in0=o2[:], in1=b2bc[:],
                                        op=mybir.AluOpType.add)
                nc.sync.dma_start(out=out_d[b * P:(b + 1) * P, :], in_=o2b[:])

    nc.compile()
    return nc


def host_prep(inputs, cfg):
    """Build per-core input maps from full inputs."""
    N, NPAD, PER, NBLK, HALF, NSUB, NCORES = (
        cfg["N"], cfg["NPAD"], cfg["PER"], cfg["NBLK"], cfg["HALF"],
        cfg["NSUB"], cfg["NCORES"])
    NEH = NSUB * P
    IDXW = NEH // 16

    x = np.asarray(inputs["x"], dtype=np.float32)
    ei = np.asarray(inputs["edge_index"], dtype=np.int64)
    W1 = np.asarray(inputs["W1"], dtype=np.float64)
    a1s = np.asarray(inputs["a1_src"], dtype=np.float64)
    a1d = np.asarray(inputs["a1_dst"], dtype=np.float64)
    b1 = np.asarray(inputs["b1"], dtype=np.float32)
    W2 = np.asarray(inputs["W2"], dtype=np.float64)
    a2s = np.asarray(inputs["a2_src"], dtype=np.float64)
    a2d = np.asarray(inputs["a2_dst"], dtype=np.float64)
    b2 = np.asarray(inputs["b2"], dtype=np.float32)

    xT = np.zeros((FIN, NPAD), dtype=np.float32)
    xT[:, :N] = x.T

    A1s = np.zeros((HD, H))
    A1d = np.zeros((HD, H))
    for hd in range(H):
        A1s[hd * C:(hd + 1) * C, hd] = a1s[hd]
        A1d[hd * C:(hd + 1) * C, hd] = a1d[hd]
    w1e = np.concatenate([W1, W1 @ A1s, W1 @ A1d], axis=1).astype(np.float32)  # [128,272]

    w2cols = np.concatenate([W2, W2 @ a2s[0][:, None], W2 @ a2d[0][:, None]],
                            axis=1)  # [HD, 4]
    w2e = w2cols.reshape(2, P, 4).transpose(1, 0, 2).reshape(P, 8).astype(NP_BF)

    loops = np.arange(N, dtype=np.int64)
    src = np.concatenate([ei[0], loops])
    dst = np.concatenate([ei[1], loops])

    in_maps = []
    for c in range(NCORES):
        lo_n, hi_n = c * PER, (c + 1) * PER
        m = (dst >= lo_n) & (dst < hi_n)
        s_c = src[m]
        d_c = dst[m] - lo_n
        blk = d_c >> 7
        dloc = d_c & 127
        halfsel = (s_c >= HALF).astype(np.int64)
        key = blk * 2 + halfsel
        order = np.argsort(key, kind="stable")
        key_s = key[order]
        cnt = np.bincount(key_s, minlength=NBLK * 2)
        assert cnt.max() <= NEH, f"bucket overflow: {cnt.max()} > {NEH}"
        starts = np.zeros(NBLK * 2, dtype=np.int64)
        starts[1:] = np.cumsum(cnt)[:-1]
        pos = np.arange(len(key_s)) - starts[key_s]
        slot = key_s * NEH + pos
        idxflat = np.zeros(NBLK * 2 * NEH, dtype=np.int16)
        dstflat = np.full(NBLK * 2 * NEH, -1.0, dtype=np.float32)
        sv = s_c[order] - halfsel[order] * HALF
        idxflat[slot] = sv.astype(np.int16)
        dstflat[slot] = dloc[order].astype(np.float32)

        idxw16 = (idxflat.reshape(NBLK * 2, NSUB * 8, 16)
                  .transpose(2, 0, 1).reshape(16, -1))
        idxw = np.tile(idxw16, (8, 1))  # replicated across the 8 Q7 cores
        dstw = (dstflat.reshape(NBLK * 2, NSUB, P).transpose(2, 0, 1)
                .reshape(P, NBLK * 2 * NSUB))

        in_maps.append({
            "xT": xT, "w1e": w1e, "w2e": w2e,
            "b1r": b1.reshape(1, HD).astype(np.float32),
            "b2r": b2.reshape(1, 2).astype(np.float32),
            "idx16": idxw, "dstf": np.ascontiguousarray(dstw),
        })
    return in_maps


_NC_CACHE = {}


def _get_nc():
    if "nc" not in _NC_CACHE:
        _NC_CACHE["nc"] = build_nc(FULL_CFG)
    return _NC_CACHE["nc"]


def kernel(**inputs):
    from concourse.bass_utils import run_bass_kernel_spmd

    nc = _get_nc()
    in_maps = host_prep(inputs, FULL_CFG)
    res = run_bass_kernel_spmd(nc, in_maps, core_ids=list(range(FULL_CFG["NCORES"])))
    out = np.concatenate([r["out"] for r in res.results])[:FULL_CFG["N"]]
    return np.ascontiguousarray(out.astype(np.float32))



# revision 11
# speedup vs baseline: 1.0430x; 1.0430x over previous
"""Self-contained Trainium2 Bass kernel for the 2-layer GAT problem.

Accepts FULL inputs, shards destination-node ranges across 8 NeuronCores
internally, and returns the FULL [50000, 2] float32 output.

v2: gather idx streams padded with -1 (Q7 auto-trims trailing negatives),
host-shipped broadcast dst rows replace on-chip transposes for S^T,
batched S build / message multiply, cheap ELU, memsets off GpSimd.
"""
import numpy as np

import concourse.bacc as bacc
import concourse.mybir as mybir
import concourse.tile as tile
from concourse.masks import make_identity

F32 = mybir.dt.float32
BF = mybir.dt.bfloat16
I16 = mybir.dt.int16
NP_BF = mybir.dt.np(BF)

H = 8       # heads
C = 32      # per-head channels
HD = H * C  # 256
FIN = 128
ELEM = 384
ELEM2 = 128
P = 128

FULL_CFG = dict(
    N=50000, NPAD=50176, PER=6272, NBLK=49, HALF=25088, NSUB=10, NCORES=8,
    XCHUNK=1024,
)


def build_nc(cfg, regs=None):
    NPAD, PER, NBLK, HALF, NSUB = (
        cfg["NPAD"], cfg["PER"], cfg["NBLK"], cfg["HALF"], cfg["NSUB"])
    NCORES = cfg["NCORES"]
    XCHUNK = cfg["XCHUNK"]
    NEH = NSUB * P                # idx slots per (block, half) gather
    IDXW = NEH // 16              # idx cols per bucket
    if regs is None:
        regs = [NEH] * (NBLK * 2)
    assert NPAD == NCORES * PER and PER == NBLK * P and NPAD % XCHUNK == 0
    assert HALF % P == 0 and 2 * HALF == NPAD

    AL = mybir.AluOpType
    AF = mybir.ActivationFunctionType

    nc = bacc.Bacc(None, target_bir_lowering=False, num_devices=NCORES)

    xT_d = nc.dram_tensor("xT", [FIN, NPAD], F32, kind="ExternalInput")
    w1e_d = nc.dram_tensor("w1e", [FIN, 280], F32, kind="ExternalInput")
    w2e_d = nc.dram_tensor("w2e", [P, 8], BF, kind="ExternalInput")
    b1_d = nc.dram_tensor("b1r", [1, HD], F32, kind="ExternalInput")
    b2_d = nc.dram_tensor("b2r", [1, 2], F32, kind="ExternalInput")
    idx_d = nc.dram_tensor("idx16", [P, NBLK * 2 * IDXW], I16, kind="ExternalInput")
    dst_d = nc.dram_tensor("dstf", [P, NBLK * 2 * NSUB], F32, kind="ExternalInput")
    dstb_d = nc.dram_tensor("dstb", [P, NBLK * 2 * NEH], BF, kind="ExternalInput")
    out_d = nc.dram_tensor("out", [PER, 2], F32, kind="ExternalOutput")

    table = nc.dram_tensor("table", [NPAD, ELEM], BF)
    t2loc = nc.dram_tensor("t2loc", [PER, ELEM2], BF)
    table2 = nc.dram_tensor("table2", [NPAD, ELEM2], BF)

    with tile.TileContext(nc) as tc:
        with (
            tc.tile_pool(name="cst", bufs=1) as cst,
            tc.tile_pool(name="xp", bufs=2) as xp,
            tc.tile_pool(name="rowp", bufs=3) as rowp,
            tc.tile_pool(name="dstbp", bufs=2) as dstbp,
            tc.tile_pool(name="wk", bufs=3) as wk,
            tc.tile_pool(name="tailp", bufs=2) as tailp,
            tc.tile_pool(name="ps", bufs=2, space="PSUM") as ps,
        ):
            # ---- constants ----
            ident_sb = cst.tile([P, P], BF)
            make_identity(nc, ident_sb[:])
            iota_i = cst.tile([P, P], I16)
            nc.gpsimd.iota(iota_i[:], pattern=[[1, P]], base=0, channel_multiplier=0)
            iota_bf = cst.tile([P, P], BF)
            nc.vector.tensor_copy(iota_bf[:], iota_i[:])
            iotp_i = cst.tile([P, 1], I16)
            nc.gpsimd.iota(iotp_i[:], pattern=[[0, 1]], base=0, channel_multiplier=1)
            iota_part = cst.tile([P, 1], F32)
            nc.vector.tensor_copy(iota_part[:], iotp_i[:])
            onesk = cst.tile([1, P], F32)
            nc.vector.memset(onesk[:], 1.0)

            w1e_sb = cst.tile([FIN, 280], F32)
            nc.sync.dma_start(out=w1e_sb[:], in_=w1e_d[:])
            w2e_sb = cst.tile([P, 2, 4], BF)
            nc.sync.dma_start(out=w2e_sb[:], in_=w2e_d[:].rearrange("p (k n) -> p k n", k=2))
            idx_sb = cst.tile([P, NBLK * 2 * IDXW], I16)
            nc.sync.dma_start(out=idx_sb[:], in_=idx_d[:])
            dst_sb = cst.tile([P, NBLK * 2 * NSUB], F32)
            nc.sync.dma_start(out=dst_sb[:], in_=dst_d[:])

            # bias broadcast rows -> [P, HD], [P, 2]
            b1r = cst.tile([1, HD], F32)
            nc.sync.dma_start(out=b1r[:], in_=b1_d[:])
            b2r = cst.tile([1, 2], F32)
            nc.sync.dma_start(out=b2r[:], in_=b2_d[:])
            bps = ps.tile([P, HD], F32, space="PSUM", tag="accum")
            nc.tensor.matmul(out=bps[:], lhsT=onesk[:], rhs=b1r[:], start=True, stop=True)
            b1bc = cst.tile([P, HD], F32)
            nc.scalar.copy(b1bc[:], bps[:])
            bps2 = ps.tile([P, 2], F32, space="PSUM", tag="accum")
            nc.tensor.matmul(out=bps2[:], lhsT=onesk[:], rhs=b2r[:], start=True, stop=True)
            b2bc = cst.tile([P, 2], F32)
            nc.scalar.copy(b2bc[:], bps2[:])

            # persistent double-buffered gather targets, memset once so that
            # stale lanes of partially-filled subtiles stay finite
            gbufs = [cst.tile([P, NSUB, ELEM], BF, name=f"gbuf{i}") for i in range(2)]
            g2bufs = [cst.tile([P, NSUB, ELEM2], BF, name=f"g2buf{i}") for i in range(2)]
            for t_ in gbufs + g2bufs:
                nc.vector.memset(t_[:], 0.0)
            # persistent layer-2 row staging: constant cols preset once
            t2rows = [cst.tile([P, ELEM2], BF, name=f"t2row{i}") for i in range(2)]
            for t_ in t2rows:
                nc.vector.memset(t_[:], 0.0)
                nc.vector.memset(t_[:, 2:3], 1.0)

            # ---- P1: node features -> table (replicated over all nodes) ----
            for ch in range(NPAD // XCHUNK):
                xc = xp.tile([FIN, XCHUNK], F32, tag="xc")
                nc.sync.dma_start(out=xc[:], in_=xT_d[:, ch * XCHUNK:(ch + 1) * XCHUNK])
                for j in range(XCHUNK // P):
                    nt = ch * (XCHUNK // P) + j
                    ph = ps.tile([P, 280], F32, space="PSUM", tag="accum")
                    nc.tensor.matmul(out=ph[:], lhsT=xc[:, j * P:(j + 1) * P],
                                     rhs=w1e_sb[:], start=True, stop=True)
                    row = rowp.tile([P, 280], BF, tag="row")
                    nc.scalar.copy(row[:], ph[:])
                    nc.vector.memset(
                        row[:, 0:264].rearrange("p (h x) -> p h x", h=H)[:, :, C:C + 1],
                        1.0)
                    nc.sync.dma_start(out=table[nt * P:(nt + 1) * P, 0:280], in_=row[:])

            # ---- adst slice for own dst range (pid ladder) ----
            adst_sb = cst.tile([P, NBLK, H], BF)
            pid = nc.sync.partition_id()
            for c in range(NCORES):
                with tc.If(pid == c):
                    nc.sync.dma_start(
                        out=adst_sb[:],
                        in_=table[c * PER:(c + 1) * PER, 272:280]
                            .rearrange("(b p) h -> p b h", p=P))

            adst2_sb = cst.tile([P, NBLK], BF)

            # ---- P2: layer-1 message passing over own dst blocks ----
            for b in range(NBLK):
                pblk = ps.tile([P, 264], F32, space="PSUM", tag="accum")
                for half in (0, 1):
                    bucket = b * 2 + half
                    c0 = bucket * NSUB
                    g = gbufs[bucket % 2]
                    nc.gpsimd.dma_gather(
                        out_ap=g[:],
                        in_ap=(table[0:HALF, :] if half == 0 else table[HALF:NPAD, :]),
                        idxs_ap=idx_sb[:, bucket * IDXW:(bucket + 1) * IDXW],
                        num_idxs=NEH, num_idxs_reg=int(regs[bucket]),
                        elem_size=ELEM, single_packet=False)
                    dstb = dstbp.tile([P, NEH], BF, tag="dstb")
                    nc.sync.dma_start(out=dstb[:],
                                      in_=dstb_d[:, bucket * NEH:(bucket + 1) * NEH])
                    S_all = wk.tile([P, NSUB, P], BF, tag="S_all", bufs=2)
                    nc.vector.tensor_tensor(
                        out=S_all[:],
                        in0=dst_sb[:, c0:c0 + NSUB].rearrange("p (t o) -> p t o", o=1)
                            .to_broadcast([P, NSUB, P]),
                        in1=iota_bf[:].rearrange("p (o e) -> p o e", o=1)
                            .to_broadcast([P, NSUB, P]),
                        op=AL.is_equal)
                    S_T = wk.tile([P, NEH], BF, tag="S_T", bufs=2)
                    nc.vector.tensor_scalar(
                        out=S_T[:], in0=dstb[:], scalar1=iota_part[:],
                        scalar2=None, op0=AL.is_equal)
                    aexp = ps.tile([P, NSUB, H], F32, space="PSUM", tag="aexp")
                    for t in range(NSUB):
                        nc.tensor.matmul(out=aexp[:, t, :],
                                         lhsT=S_T[:, t * P:(t + 1) * P],
                                         rhs=adst_sb[:, b, :], start=True, stop=True)
                    logits = wk.tile([P, NSUB, H], F32, tag="logits")
                    nc.vector.tensor_tensor(out=logits[:], in0=g[:, :, 264:272],
                                            in1=aexp[:], op=AL.add)
                    e1 = wk.tile([P, NSUB, H], F32, tag="e1")
                    nc.scalar.activation(e1[:], logits[:], AF.Exp)
                    e2 = wk.tile([P, NSUB, H], F32, tag="e2")
                    nc.scalar.activation(e2[:], logits[:], AF.Exp, scale=0.2)
                    wt = wk.tile([P, NSUB, H], BF, tag="wt")
                    nc.vector.tensor_tensor(out=wt[:], in0=e1[:], in1=e2[:],
                                            op=AL.max)
                    msgall = wk.tile([P, NSUB, 264], BF, tag="msgall")
                    nc.vector.tensor_tensor(
                        out=msgall[:].rearrange("p t (h x) -> p t h x", h=H),
                        in0=g[:, :, 0:264].rearrange("p t (h x) -> p t h x", h=H),
                        in1=wt[:, :, :, None]
                            .to_broadcast([P, NSUB, H, C + 1]),
                        op=AL.mult)
                    for t in range(NSUB):
                        nc.tensor.matmul(out=pblk[:], lhsT=S_all[:, t, :],
                                         rhs=msgall[:, t, :],
                                         start=(half == 0 and t == 0),
                                         stop=(half == 1 and t == NSUB - 1))
                # tail: normalize + bias + ELU -> h2 -> t2loc rows
                pb3 = pblk[:].rearrange("p (h x) -> p h x", h=H)
                srec = tailp.tile([P, H], F32, tag="srec")
                nc.vector.tensor_scalar(
                    out=srec[:], in0=pb3[:, :, C:C + 1].rearrange("p h x -> p (h x)"),
                    scalar1=1e-16, scalar2=None, op0=AL.add)
                rec = tailp.tile([P, H], F32, tag="rec")
                nc.vector.reciprocal(rec[:], srec[:])
                out1 = tailp.tile([P, HD], F32, tag="out1")
                nc.vector.tensor_tensor(
                    out=out1[:].rearrange("p (h c) -> p h c", h=H),
                    in0=pb3[:, :, 0:C],
                    in1=rec[:, :, None].to_broadcast([P, H, C]),
                    op=AL.mult)
                v = tailp.tile([P, HD], F32, tag="v")
                nc.vector.tensor_tensor(out=v[:], in0=out1[:], in1=b1bc[:],
                                        op=AL.add)
                ev = tailp.tile([P, HD], F32, tag="ev")
                nc.scalar.activation(ev[:], v[:], AF.Exp)
                em = tailp.tile([P, HD], F32, tag="em")
                nc.vector.tensor_scalar(out=em[:], in0=ev[:], scalar1=1.0,
                                        scalar2=None, op0=AL.subtract)
                em2 = tailp.tile([P, HD], F32, tag="em2")
                nc.vector.tensor_scalar(out=em2[:], in0=em[:], scalar1=0.0,
                                        scalar2=None, op0=AL.min)
                pp = tailp.tile([P, HD], F32, tag="pp")
                nc.scalar.activation(pp[:], v[:], AF.Relu)
                elu = tailp.tile([P, HD], BF, tag="elu")
                nc.vector.tensor_tensor(out=elu[:], in0=em2[:], in1=pp[:],
                                        op=AL.add)
                eT_sb = tailp.tile([P, 2, P], BF, tag="eT_sb")
                ph2 = ps.tile([P, 4], F32, space="PSUM", tag="tail")
                for k in range(2):
                    eT_ps = ps.tile([P, P], BF, space="PSUM", tag="tps")
                    nc.tensor.transpose(eT_ps[:], elu[:, k * P:(k + 1) * P], ident_sb[:])
                    nc.scalar.copy(eT_sb[:, k, :], eT_ps[:])
                for k in range(2):
                    nc.tensor.matmul(out=ph2[:], lhsT=eT_sb[:, k, :], rhs=w2e_sb[:, k, :],
                                     start=(k == 0), stop=(k == 1))
                t2row = t2rows[b % 2]
                nc.scalar.copy(t2row[:, 0:2], ph2[:, 0:2])
                nc.scalar.copy(t2row[:, 3:5], ph2[:, 2:4])
                nc.sync.dma_start(out=t2loc[b * P:(b + 1) * P, :], in_=t2row[:])
                nc.scalar.copy(adst2_sb[:, b:b + 1], ph2[:, 3:4])

            # ---- AllGather layer-2 node table ----
            nc.gpsimd.collective_compute(
                "AllGather", AL.bypass,
                replica_groups=[list(range(NCORES))],
                ins=[t2loc[:]], outs=[table2[:]])

            # ---- P3: layer-2 message passing ----
            for b in range(NBLK):
                p2s = ps.tile([P, 3], F32, space="PSUM", tag="accum")
                for half in (0, 1):
                    bucket = b * 2 + half
                    c0 = bucket * NSUB
                    g2 = g2bufs[bucket % 2]
                    nc.gpsimd.dma_gather(
                        out_ap=g2[:],
                        in_ap=(table2[0:HALF, :] if half == 0 else table2[HALF:NPAD, :]),
                        idxs_ap=idx_sb[:, bucket * IDXW:(bucket + 1) * IDXW],
                        num_idxs=NEH, num_idxs_reg=int(regs[bucket]),
                        elem_size=ELEM2, single_packet=False)
                    dstb2 = dstbp.tile([P, NEH], BF, tag="dstb")
                    nc.sync.dma_start(out=dstb2[:],
                                      in_=dstb_d[:, bucket * NEH:(bucket + 1) * NEH])
                    S2_all = wk.tile([P, NSUB, P], BF, tag="S_all", bufs=2)
                    nc.vector.tensor_tensor(
                        out=S2_all[:],
                        in0=dst_sb[:, c0:c0 + NSUB].rearrange("p (t o) -> p t o", o=1)
                            .to_broadcast([P, NSUB, P]),
                        in1=iota_bf[:].rearrange("p (o e) -> p o e", o=1)
                            .to_broadcast([P, NSUB, P]),
                        op=AL.is_equal)
                    S2_T = wk.tile([P, NEH], BF, tag="S_T", bufs=2)
                    nc.vector.tensor_scalar(
                        out=S2_T[:], in0=dstb2[:], scalar1=iota_part[:],
                        scalar2=None, op0=AL.is_equal)
                    a2e = ps.tile([P, NSUB], F32, space="PSUM", tag="aexp")
                    for t in range(NSUB):
                        nc.tensor.matmul(out=a2e[:, t:t + 1],
                                         lhsT=S2_T[:, t * P:(t + 1) * P],
                                         rhs=adst2_sb[:, b:b + 1], start=True, stop=True)
                    lg2 = wk.tile([P, NSUB], F32, tag="logits2")
                    nc.vector.tensor_tensor(
                        out=lg2[:], in0=g2[:, :, 3:4].rearrange("p t x -> p (t x)"),
                        in1=a2e[:], op=AL.add)
                    f1 = wk.tile([P, NSUB], F32, tag="f1")
                    nc.scalar.activation(f1[:], lg2[:], AF.Exp)
                    f2 = wk.tile([P, NSUB], F32, tag="f2")
                    nc.scalar.activation(f2[:], lg2[:], AF.Exp, scale=0.2)
                    w2t = wk.tile([P, NSUB], BF, tag="w2t")
                    nc.vector.tensor_tensor(out=w2t[:], in0=f1[:], in1=f2[:],
                                            op=AL.max)
                    msg2all = wk.tile([P, NSUB, 3], BF, tag="msg2all")
                    nc.vector.tensor_tensor(
                        out=msg2all[:], in0=g2[:, :, 0:3],
                        in1=w2t[:].rearrange("p (t o) -> p t o", o=1).to_broadcast([P, NSUB, 3]),
                        op=AL.mult)
                    for t in range(NSUB):
                        nc.tensor.matmul(out=p2s[:], lhsT=S2_all[:, t, :],
                                         rhs=msg2all[:, t, :],
                                         start=(half == 0 and t == 0),
                                         stop=(half == 1 and t == NSUB - 1))
                s2r = tailp.tile([P, 1], F32, tag="s2r")
                nc.vector.tensor_scalar(out=s2r[:], in0=p2s[:, 2:3], scalar1=1e-16,
                                        scalar2=None, op0=AL.add)
                rec2 = tailp.tile([P, 1], F32, tag="rec2")
                nc.vector.reciprocal(rec2[:], s2r[:])
                o2 = tailp.tile([P, 2], F32, tag="o2")
                nc.vector.tensor_tensor(out=o2[:], in0=p2s[:, 0:2],
                                        in1=rec2[:].to_broadcast([P, 2]),
                                        op=AL.mult)
                o2b = tailp.tile([P, 2], F32, tag="o2b")
                nc.vector.tensor_tensor(out=o2b[:], in0=o2[:], in1=b2bc[:],
                                        op=AL.add)
                nc.sync.dma_start(out=out_d[b * P:(b + 1) * P, :], in_=o2b[:])

    nc.compile()
    return nc


def host_prep(inputs, cfg):
    """Build per-core input maps from full inputs."""
    N, NPAD, PER, NBLK, HALF, NSUB, NCORES = (
        cfg["N"], cfg["NPAD"], cfg["PER"], cfg["NBLK"], cfg["HALF"],
        cfg["NSUB"], cfg["NCORES"])
    NEH = NSUB * P
    IDXW = NEH // 16

    x = np.asarray(inputs["x"], dtype=np.float32)
    ei = np.asarray(inputs["edge_index"], dtype=np.int64)
    W1 = np.asarray(inputs["W1"], dtype=np.float64)
    a1s = np.asarray(inputs["a1_src"], dtype=np.float64)
    a1d = np.asarray(inputs["a1_dst"], dtype=np.float64)
    b1 = np.asarray(inputs["b1"], dtype=np.float32)
    W2 = np.asarray(inputs["W2"], dtype=np.float64)
    a2s = np.asarray(inputs["a2_src"], dtype=np.float64)
    a2d = np.asarray(inputs["a2_dst"], dtype=np.float64)
    b2 = np.asarray(inputs["b2"], dtype=np.float32)

    xT = np.zeros((FIN, NPAD), dtype=np.float32)
    xT[:, :N] = x.T

    A1s = np.zeros((HD, H))
    A1d = np.zeros((HD, H))
    for hd in range(H):
        A1s[hd * C:(hd + 1) * C, hd] = a1s[hd]
        A1d[hd * C:(hd + 1) * C, hd] = a1d[hd]
    # column layout: 8 heads x [32 W1-cols, 1 zero col] then ls(8), ld(8)
    w1e = np.zeros((FIN, 280), dtype=np.float32)
    for hd in range(H):
        w1e[:, hd * 33:hd * 33 + 32] = W1[:, hd * 32:(hd + 1) * 32]
    w1e[:, 264:272] = W1 @ A1s
    w1e[:, 272:280] = W1 @ A1d

    w2cols = np.concatenate([W2, W2 @ a2s[0][:, None], W2 @ a2d[0][:, None]],
                            axis=1)  # [HD, 4]
    w2e = w2cols.reshape(2, P, 4).transpose(1, 0, 2).reshape(P, 8).astype(NP_BF)

    loops = np.arange(N, dtype=np.int64)
    src = np.concatenate([ei[0], loops])
    dst = np.concatenate([ei[1], loops])

    # Pass 1: per-core bucket assignment + counts. The gather's descriptor-ring
    # reservation (decode, from num_idxs_reg) must exactly match what the Q7
    # ucode pushes after trailing-negative trimming, and the program is shared
    # across cores, so the trim point is the max count over cores, rounded up
    # to the 128-idx reservation granularity. Slots [actual, reg) fetch row 0
    # (masked via dst=-1); slots [reg, NEH) are -1 and get trimmed.
    cnts = np.zeros((NCORES, NBLK * 2), dtype=np.int64)
    percore = []
    for c in range(NCORES):
        lo_n, hi_n = c * PER, (c + 1) * PER
        m = (dst >= lo_n) & (dst < hi_n)
        s_c = src[m]
        d_c = dst[m] - lo_n
        blk = d_c >> 7
        dloc = d_c & 127
        halfsel = (s_c >= HALF).astype(np.int64)
        key = blk * 2 + halfsel
        order = np.argsort(key, kind="stable")
        key_s = key[order]
        cnt = np.bincount(key_s, minlength=NBLK * 2)
        assert cnt.max() <= NEH, f"bucket overflow: {cnt.max()} > {NEH}"
        cnts[c] = cnt
        starts = np.zeros(NBLK * 2, dtype=np.int64)
        starts[1:] = np.cumsum(cnt)[:-1]
        pos = np.arange(len(key_s)) - starts[key_s]
        slot = key_s * NEH + pos
        sv = s_c[order] - halfsel[order] * HALF
        percore.append((slot, sv, dloc[order]))

    regs = np.minimum((cnts.max(axis=0) + 127) // 128 * 128, NEH)
    regmask = (np.arange(NEH)[None, :] < regs[:, None]).reshape(-1)  # [NBLK*2*NEH]

    in_maps = []
    for c in range(NCORES):
        slot, sv, dl = percore[c]
        idxflat = np.full(NBLK * 2 * NEH, -1, dtype=np.int16)
        idxflat[regmask] = 0
        dstflat = np.full(NBLK * 2 * NEH, -1.0, dtype=np.float32)
        idxflat[slot] = sv.astype(np.int16)
        dstflat[slot] = dl.astype(np.float32)

        idxw16 = (idxflat.reshape(NBLK * 2, NSUB * 8, 16)
                  .transpose(2, 0, 1).reshape(16, -1))
        idxw = np.tile(idxw16, (8, 1))  # replicated across the 8 Q7 cores
        dstw = (dstflat.reshape(NBLK * 2, NSUB, P).transpose(2, 0, 1)
                .reshape(P, NBLK * 2 * NSUB))
        dstb = np.ascontiguousarray(
            np.broadcast_to(dstflat.astype(NP_BF)[None, :], (P, NBLK * 2 * NEH)))

        in_maps.append({
            "xT": xT, "w1e": w1e, "w2e": w2e,
            "b1r": b1.reshape(1, HD).astype(np.float32),
            "b2r": b2.reshape(1, 2).astype(np.float32),
            "idx16": idxw, "dstf": np.ascontiguousarray(dstw),
            "dstb": dstb,
        })
    return in_maps, regs


_NC_CACHE = {}


def _get_nc(regs):
    key = tuple(int(r) for r in regs)
    if _NC_CACHE.get("key") != key:
        _NC_CACHE["nc"] = build_nc(FULL_CFG, regs)
        _NC_CACHE["key"] = key
    return _NC_CACHE["nc"]


def kernel(**inputs):
    from concourse.bass_utils import run_bass_kernel_spmd

    in_maps, regs = host_prep(inputs, FULL_CFG)
    nc = _get_nc(regs)
    res = run_bass_kernel_spmd(nc, in_maps, core_ids=list(range(FULL_CFG["NCORES"])))
    out = np.concatenate([r["out"] for r in res.results])[:FULL_CFG["N"]]
    return np.ascontiguousarray(out.astype(np.float32))


# revision 14
# speedup vs baseline: 1.4276x; 1.3688x over previous
"""Self-contained Trainium2 Bass kernel for the 2-layer GAT problem.

Accepts FULL inputs, shards destination-node ranges across 8 NeuronCores
internally, and returns the FULL [50000, 2] float32 output.

v2: gather idx streams padded with -1 (Q7 auto-trims trailing negatives),
host-shipped broadcast dst rows replace on-chip transposes for S^T,
batched S build / message multiply, cheap ELU, memsets off GpSimd.
"""
import numpy as np

import concourse.bacc as bacc
import concourse.mybir as mybir
import concourse.tile as tile
from concourse.masks import make_identity

F32 = mybir.dt.float32
BF = mybir.dt.bfloat16
I16 = mybir.dt.int16
NP_BF = mybir.dt.np(BF)

H = 8       # heads
C = 32      # per-head channels
HD = H * C  # 256
FIN = 128
ELEM = 384
ELEM2 = 128
P = 128

FULL_CFG = dict(
    N=50000, NPAD=50176, PER=6272, NBLK=49, HALF=25088, NSUB=10, NCORES=8,
    XCHUNK=1024,
)


def build_nc(cfg, regs=None):
    NPAD, PER, NBLK, HALF, NSUB = (
        cfg["NPAD"], cfg["PER"], cfg["NBLK"], cfg["HALF"], cfg["NSUB"])
    NCORES = cfg["NCORES"]
    XCHUNK = cfg["XCHUNK"]
    NEH = NSUB * P                # idx slots per (block, half) gather
    IDXW = NEH // 16              # idx cols per bucket
    if regs is None:
        regs = [NEH] * (NBLK * 2)
    assert NPAD == NCORES * PER and PER == NBLK * P and NPAD % XCHUNK == 0
    assert HALF % P == 0 and 2 * HALF == NPAD

    AL = mybir.AluOpType
    AF = mybir.ActivationFunctionType

    nc = bacc.Bacc(None, target_bir_lowering=False, num_devices=NCORES)

    xT_d = nc.dram_tensor("xT", [FIN, NPAD], BF, kind="ExternalInput")
    w1e_d = nc.dram_tensor("w1e", [FIN, 280], BF, kind="ExternalInput")
    w2e_d = nc.dram_tensor("w2e", [P, 8], BF, kind="ExternalInput")
    b1_d = nc.dram_tensor("b1r", [1, HD], F32, kind="ExternalInput")
    b2_d = nc.dram_tensor("b2r", [1, 2], F32, kind="ExternalInput")
    idx_d = nc.dram_tensor("idx16", [P, NBLK * 2 * IDXW], I16, kind="ExternalInput")
    dst_d = nc.dram_tensor("dstf", [P, NBLK * 2 * NSUB], F32, kind="ExternalInput")
    dstb_d = nc.dram_tensor("dstb", [P, NBLK * 2 * NEH], BF, kind="ExternalInput")
    out_d = nc.dram_tensor("out", [PER, 2], F32, kind="ExternalOutput")

    table = nc.dram_tensor("table", [NPAD, ELEM], BF)
    t2loc = nc.dram_tensor("t2loc", [PER, ELEM2], BF)
    table2 = nc.dram_tensor("table2", [NPAD, ELEM2], BF)

    with tile.TileContext(nc) as tc:
        with (
            tc.tile_pool(name="cst", bufs=1) as cst,
            tc.tile_pool(name="xp", bufs=2) as xp,
            tc.tile_pool(name="rowp", bufs=3) as rowp,
            tc.tile_pool(name="dstbp", bufs=3) as dstbp,
            tc.tile_pool(name="wk", bufs=3) as wk,
            tc.tile_pool(name="tailp", bufs=2) as tailp,
            tc.tile_pool(name="ps", bufs=2, space="PSUM") as ps,
        ):
            # ---- constants ----
            ident_sb = cst.tile([P, P], BF)
            make_identity(nc, ident_sb[:])
            iota_i = cst.tile([P, P], I16)
            nc.gpsimd.iota(iota_i[:], pattern=[[1, P]], base=0, channel_multiplier=0)
            iota_bf = cst.tile([P, P], BF)
            nc.vector.tensor_copy(iota_bf[:], iota_i[:])
            iotp_i = cst.tile([P, 1], I16)
            nc.gpsimd.iota(iotp_i[:], pattern=[[0, 1]], base=0, channel_multiplier=1)
            iota_part = cst.tile([P, 1], F32)
            nc.vector.tensor_copy(iota_part[:], iotp_i[:])
            onesk = cst.tile([1, P], F32)
            nc.vector.memset(onesk[:], 1.0)
            ones_hd = cst.tile([P, HD], F32)
            nc.vector.memset(ones_hd[:], 1.0)
            zeros_hd = cst.tile([P, HD], F32)
            nc.vector.memset(zeros_hd[:], 0.0)

            w1e_sb = cst.tile([FIN, 280], BF)
            nc.sync.dma_start(out=w1e_sb[:], in_=w1e_d[:])
            w2e_sb = cst.tile([P, 2, 4], BF)
            nc.sync.dma_start(out=w2e_sb[:], in_=w2e_d[:].rearrange("p (k n) -> p k n", k=2))
            idx_sb = cst.tile([P, NBLK * 2 * IDXW], I16)
            nc.sync.dma_start(out=idx_sb[:], in_=idx_d[:])
            dst_sb = cst.tile([P, NBLK * 2 * NSUB], F32)
            nc.sync.dma_start(out=dst_sb[:], in_=dst_d[:])

            # bias broadcast rows -> [P, HD], [P, 2]
            b1r = cst.tile([1, HD], F32)
            nc.sync.dma_start(out=b1r[:], in_=b1_d[:])
            b2r = cst.tile([1, 2], F32)
            nc.sync.dma_start(out=b2r[:], in_=b2_d[:])
            bps = ps.tile([P, HD], F32, space="PSUM", tag="accum")
            nc.tensor.matmul(out=bps[:], lhsT=onesk[:], rhs=b1r[:], start=True, stop=True)
            b1bc = cst.tile([P, HD], F32)
            nc.scalar.copy(b1bc[:], bps[:])
            bps2 = ps.tile([P, 2], F32, space="PSUM", tag="accum")
            nc.tensor.matmul(out=bps2[:], lhsT=onesk[:], rhs=b2r[:], start=True, stop=True)
            b2bc = cst.tile([P, 2], F32)
            nc.scalar.copy(b2bc[:], bps2[:])

            # persistent double-buffered gather targets, memset once so that
            # stale lanes of partially-filled subtiles stay finite
            gbufs = [cst.tile([P, NSUB, ELEM], BF, name=f"gbuf{i}") for i in range(4)]
            g2bufs = [cst.tile([P, NSUB, ELEM2], BF, name=f"g2buf{i}") for i in range(4)]
            for t_ in gbufs + g2bufs:
                nc.vector.memset(t_[:], 0.0)
            # persistent layer-2 row staging: constant cols preset once
            t2rows = [cst.tile([P, ELEM2], BF, name=f"t2row{i}") for i in range(2)]
            for t_ in t2rows:
                nc.vector.memset(t_[:], 0.0)
                nc.vector.memset(t_[:, 2:3], 1.0)

            # ---- P1: node features -> table (replicated over all nodes) ----
            JT = XCHUNK // P
            stages = [cst.tile([P, JT, ELEM], BF, name=f"stage{i}") for i in range(2)]
            for t_ in stages:
                nc.vector.memset(t_[:, :, 280:ELEM], 0.0)
            for ch in range(NPAD // XCHUNK):
                xc = xp.tile([FIN, XCHUNK], BF, tag="xc")
                nc.sync.dma_start(out=xc[:], in_=xT_d[:, ch * XCHUNK:(ch + 1) * XCHUNK])
                stage = stages[ch % 2]
                for j in range(JT):
                    ph = ps.tile([P, 280], F32, space="PSUM", tag="p1")
                    nc.tensor.matmul(out=ph[:], lhsT=xc[:, j * P:(j + 1) * P],
                                     rhs=w1e_sb[:], start=True, stop=True)
                    nc.scalar.copy(stage[:, j, 0:280], ph[:])
                    nc.vector.memset(
                        stage[:, j, 0:264].rearrange("p (h x) -> p h x", h=H)[:, :, C:C + 1],
                        1.0)
                nc.sync.dma_start(
                    out=table[ch * XCHUNK:(ch + 1) * XCHUNK, :]
                        .rearrange("(j p) e -> p j e", p=P),
                    in_=stage[:])

            # ---- adst slice for own dst range (pid ladder) ----
            adst_sb = cst.tile([P, NBLK, H], BF)
            pid = nc.sync.partition_id()
            for c in range(NCORES):
                with tc.If(pid == c):
                    nc.sync.dma_start(
                        out=adst_sb[:],
                        in_=table[c * PER:(c + 1) * PER, 272:280]
                            .rearrange("(b p) h -> p b h", p=P))

            adst2_sb = cst.tile([P, NBLK], BF)

            # ---- P2: layer-1 message passing over own dst blocks ----
            for b in range(NBLK):
                pblk = ps.tile([P, 264], F32, space="PSUM", tag="accum")
                for half in (0, 1):
                    bucket = b * 2 + half
                    c0 = bucket * NSUB
                    REG = int(regs[bucket])
                    nsb = REG // P
                    g = gbufs[bucket % 4]
                    nc.gpsimd.dma_gather(
                        out_ap=g[:, 0:nsb, :],
                        in_ap=(table[0:HALF, :] if half == 0 else table[HALF:NPAD, :]),
                        idxs_ap=idx_sb[:, bucket * IDXW:(bucket + 1) * IDXW],
                        num_idxs=REG, num_idxs_reg=REG,
                        elem_size=ELEM, single_packet=False)
                    nsb1 = int(regs[b * 2 + 1]) // P
                    dstb = dstbp.tile([P, NEH], BF, tag="dstb")
                    nc.sync.dma_start(out=dstb[:, 0:REG],
                                      in_=dstb_d[:, bucket * NEH:bucket * NEH + REG])
                    S_all = wk.tile([P, NSUB, P], BF, tag="S_all", bufs=2)
                    nc.vector.tensor_tensor(
                        out=S_all[:, 0:nsb, :],
                        in0=dst_sb[:, c0:c0 + nsb].rearrange("p (t o) -> p t o", o=1)
                            .to_broadcast([P, nsb, P]),
                        in1=iota_bf[:].rearrange("p (o e) -> p o e", o=1)
                            .to_broadcast([P, nsb, P]),
                        op=AL.is_equal)
                    S_T = wk.tile([P, NEH], BF, tag="S_T", bufs=2)
                    nc.vector.tensor_scalar(
                        out=S_T[:, 0:REG], in0=dstb[:, 0:REG], scalar1=iota_part[:],
                        scalar2=None, op0=AL.is_equal)
                    aexp = ps.tile([P, NSUB, H], F32, space="PSUM", tag="aexp")
                    for t in range(nsb):
                        nc.tensor.matmul(out=aexp[:, t, :],
                                         lhsT=S_T[:, t * P:(t + 1) * P],
                                         rhs=adst_sb[:, b, :], start=True, stop=True)
                    logits = wk.tile([P, NSUB, H], F32, tag="logits")
                    nc.vector.tensor_tensor(out=logits[:, 0:nsb, :],
                                            in0=g[:, 0:nsb, 264:272],
                                            in1=aexp[:, 0:nsb, :], op=AL.add)
                    e1 = wk.tile([P, NSUB, H], F32, tag="e1")
                    nc.scalar.activation(e1[:, 0:nsb, :], logits[:, 0:nsb, :], AF.Exp)
                    e2 = wk.tile([P, NSUB, H], F32, tag="e2")
                    nc.scalar.activation(e2[:, 0:nsb, :], logits[:, 0:nsb, :], AF.Exp,
                                         scale=0.2)
                    wt = wk.tile([P, NSUB, H], BF, tag="wt")
                    nc.vector.tensor_tensor(out=wt[:, 0:nsb, :], in0=e1[:, 0:nsb, :],
                                            in1=e2[:, 0:nsb, :], op=AL.max)
                    msgall = wk.tile([P, NSUB, 264], BF, tag="msgall")
                    nc.vector.tensor_tensor(
                        out=msgall[:, 0:nsb, :].rearrange("p t (h x) -> p t h x", h=H),
                        in0=g[:, 0:nsb, 0:264].rearrange("p t (h x) -> p t h x", h=H),
                        in1=wt[:, 0:nsb, :, None]
                            .to_broadcast([P, nsb, H, C + 1]),
                        op=AL.mult)
                    for t in range(nsb):
                        nc.tensor.matmul(out=pblk[:], lhsT=S_all[:, t, :],
                                         rhs=msgall[:, t, :],
                                         start=(half == 0 and t == 0),
                                         stop=(half == 1 and t == nsb1 - 1))
                # tail: normalize + bias + ELU -> h2 -> t2loc rows
                pb3 = pblk[:].rearrange("p (h x) -> p h x", h=H)
                srec = tailp.tile([P, H], F32, tag="srec")
                nc.vector.tensor_scalar(
                    out=srec[:], in0=pb3[:, :, C:C + 1].rearrange("p h x -> p (h x)"),
                    scalar1=1e-16, scalar2=None, op0=AL.add)
                rec = tailp.tile([P, H], F32, tag="rec")
                nc.vector.reciprocal(rec[:], srec[:])
                out1 = tailp.tile([P, HD], F32, tag="out1")
                nc.vector.tensor_tensor(
                    out=out1[:].rearrange("p (h c) -> p h c", h=H),
                    in0=pb3[:, :, 0:C],
                    in1=rec[:, :, None].to_broadcast([P, H, C]),
                    op=AL.mult)
                v = tailp.tile([P, HD], F32, tag="v")
                nc.vector.tensor_tensor(out=v[:], in0=out1[:], in1=b1bc[:],
                                        op=AL.add)
                ev = tailp.tile([P, HD], F32, tag="ev")
                nc.scalar.activation(ev[:], v[:], AF.Exp)
                em = tailp.tile([P, HD], F32, tag="em")
                nc.vector.tensor_tensor(out=em[:], in0=ev[:], in1=ones_hd[:],
                                        op=AL.subtract)
                em2 = tailp.tile([P, HD], F32, tag="em2")
                nc.vector.tensor_tensor(out=em2[:], in0=em[:], in1=zeros_hd[:],
                                        op=AL.min)
                pp = tailp.tile([P, HD], F32, tag="pp")
                nc.scalar.activation(pp[:], v[:], AF.Relu)
                elu = tailp.tile([P, HD], BF, tag="elu")
                nc.vector.tensor_tensor(out=elu[:], in0=em2[:], in1=pp[:],
                                        op=AL.add)
                eT_sb = tailp.tile([P, 2, P], BF, tag="eT_sb")
                ph2 = ps.tile([P, 4], F32, space="PSUM", tag="tail", bufs=1)
                for k in range(2):
                    eT_ps = ps.tile([P, P], BF, space="PSUM", tag="tps", bufs=1)
                    nc.tensor.transpose(eT_ps[:], elu[:, k * P:(k + 1) * P], ident_sb[:])
                    nc.scalar.copy(eT_sb[:, k, :], eT_ps[:])
                for k in range(2):
                    nc.tensor.matmul(out=ph2[:], lhsT=eT_sb[:, k, :], rhs=w2e_sb[:, k, :],
                                     start=(k == 0), stop=(k == 1))
                t2row = t2rows[b % 2]
                nc.scalar.copy(t2row[:, 0:2], ph2[:, 0:2])
                nc.scalar.copy(t2row[:, 3:5], ph2[:, 2:4])
                nc.sync.dma_start(out=t2loc[b * P:(b + 1) * P, :], in_=t2row[:])
                nc.scalar.copy(adst2_sb[:, b:b + 1], ph2[:, 3:4])

            # ---- AllGather layer-2 node table ----
            nc.gpsimd.collective_compute(
                "AllGather", AL.bypass,
                replica_groups=[list(range(NCORES))],
                ins=[t2loc[:]], outs=[table2[:]])

            # ---- P3: layer-2 message passing ----
            for b in range(NBLK):
                p2s = ps.tile([P, 3], F32, space="PSUM", tag="accum")
                for half in (0, 1):
                    bucket = b * 2 + half
                    c0 = bucket * NSUB
                    REG = int(regs[bucket])
                    nsb = REG // P
                    g2 = g2bufs[bucket % 4]
                    nc.gpsimd.dma_gather(
                        out_ap=g2[:, 0:nsb, :],
                        in_ap=(table2[0:HALF, :] if half == 0 else table2[HALF:NPAD, :]),
                        idxs_ap=idx_sb[:, bucket * IDXW:(bucket + 1) * IDXW],
                        num_idxs=REG, num_idxs_reg=REG,
                        elem_size=ELEM2, single_packet=False)
                    nsb1 = int(regs[b * 2 + 1]) // P
                    dstb2 = dstbp.tile([P, NEH], BF, tag="dstb")
                    nc.sync.dma_start(out=dstb2[:, 0:REG],
                                      in_=dstb_d[:, bucket * NEH:bucket * NEH + REG])
                    S2_all = wk.tile([P, NSUB, P], BF, tag="S_all", bufs=2)
                    nc.vector.tensor_tensor(
                        out=S2_all[:, 0:nsb, :],
                        in0=dst_sb[:, c0:c0 + nsb].rearrange("p (t o) -> p t o", o=1)
                            .to_broadcast([P, nsb, P]),
                        in1=iota_bf[:].rearrange("p (o e) -> p o e", o=1)
                            .to_broadcast([P, nsb, P]),
                        op=AL.is_equal)
                    S2_T = wk.tile([P, NEH], BF, tag="S_T", bufs=2)
                    nc.vector.tensor_scalar(
                        out=S2_T[:, 0:REG], in0=dstb2[:, 0:REG], scalar1=iota_part[:],
                        scalar2=None, op0=AL.is_equal)
                    a2e = ps.tile([P, NSUB], F32, space="PSUM", tag="aexp")
                    for t in range(nsb):
                        nc.tensor.matmul(out=a2e[:, t:t + 1],
                                         lhsT=S2_T[:, t * P:(t + 1) * P],
                                         rhs=adst2_sb[:, b:b + 1], start=True, stop=True)
                    lg2 = wk.tile([P, NSUB], F32, tag="logits2")
                    nc.vector.tensor_tensor(
                        out=lg2[:, 0:nsb],
                        in0=g2[:, 0:nsb, 3:4].rearrange("p t x -> p (t x)"),
                        in1=a2e[:, 0:nsb], op=AL.add)
                    f1 = wk.tile([P, NSUB], F32, tag="f1")
                    nc.scalar.activation(f1[:, 0:nsb], lg2[:, 0:nsb], AF.Exp)
                    f2 = wk.tile([P, NSUB], F32, tag="f2")
                    nc.scalar.activation(f2[:, 0:nsb], lg2[:, 0:nsb], AF.Exp, scale=0.2)
                    w2t = wk.tile([P, NSUB], BF, tag="w2t")
                    nc.vector.tensor_tensor(out=w2t[:, 0:nsb], in0=f1[:, 0:nsb],
                                            in1=f2[:, 0:nsb], op=AL.max)
                    msg2all = wk.tile([P, NSUB, 3], BF, tag="msg2all")
                    nc.vector.tensor_tensor(
                        out=msg2all[:, 0:nsb, :], in0=g2[:, 0:nsb, 0:3],
                        in1=w2t[:, 0:nsb].rearrange("p (t o) -> p t o", o=1)
                            .to_broadcast([P, nsb, 3]),
                        op=AL.mult)
                    for t in range(nsb):
                        nc.tensor.matmul(out=p2s[:], lhsT=S2_all[:, t, :],
                                         rhs=msg2all[:, t, :],
                                         start=(half == 0 and t == 0),
                                         stop=(half == 1 and t == nsb1 - 1))
                s2r = tailp.tile([P, 1], F32, tag="s2r")
                nc.vector.tensor_scalar(out=s2r[:], in0=p2s[:, 2:3], scalar1=1e-16,
                                        scalar2=None, op0=AL.add)
                rec2 = tailp.tile([P, 1], F32, tag="rec2")
                nc.vector.reciprocal(rec2[:], s2r[:])
                o2 = tailp.tile([P, 2], F32, tag="o2")
                nc.vector.tensor_tensor(out=o2[:], in0=p2s[:, 0:2],
                                        in1=rec2[:].to_broadcast([P, 2]),
                                        op=AL.mult)
                o2b = tailp.tile([P, 2], F32, tag="o2b")
                nc.vector.tensor_tensor(out=o2b[:], in0=o2[:], in1=b2bc[:],
                                        op=AL.add)
                nc.sync.dma_start(out=out_d[b * P:(b + 1) * P, :], in_=o2b[:])

    nc.compile()
    return nc


def host_prep(inputs, cfg):
    """Build per-core input maps from full inputs."""
    N, NPAD, PER, NBLK, HALF, NSUB, NCORES = (
        cfg["N"], cfg["NPAD"], cfg["PER"], cfg["NBLK"], cfg["HALF"],
        cfg["NSUB"], cfg["NCORES"])
    NEH = NSUB * P
    IDXW = NEH // 16

    x = np.asarray(inputs["x"], dtype=np.float32)
    ei = np.asarray(inputs["edge_index"], dtype=np.int64)
    W1 = np.asarray(inputs["W1"], dtype=np.float64)
    a1s = np.asarray(inputs["a1_src"], dtype=np.float64)
    a1d = np.asarray(inputs["a1_dst"], dtype=np.float64)
    b1 = np.asarray(inputs["b1"], dtype=np.float32)
    W2 = np.asarray(inputs["W2"], dtype=np.float64)
    a2s = np.asarray(inputs["a2_src"], dtype=np.float64)
    a2d = np.asarray(inputs["a2_dst"], dtype=np.float64)
    b2 = np.asarray(inputs["b2"], dtype=np.float32)

    xT = np.zeros((FIN, NPAD), dtype=NP_BF)
    xT[:, :N] = x.T.astype(NP_BF)

    A1s = np.zeros((HD, H))
    A1d = np.zeros((HD, H))
    for hd in range(H):
        A1s[hd * C:(hd + 1) * C, hd] = a1s[hd]
        A1d[hd * C:(hd + 1) * C, hd] = a1d[hd]
    # column layout: 8 heads x [32 W1-cols, 1 zero col] then ls(8), ld(8)
    w1e = np.zeros((FIN, 280), dtype=np.float64)
    for hd in range(H):
        w1e[:, hd * 33:hd * 33 + 32] = W1[:, hd * 32:(hd + 1) * 32]
    w1e[:, 264:272] = W1 @ A1s
    w1e[:, 272:280] = W1 @ A1d
    w1e = w1e.astype(NP_BF)

    w2cols = np.concatenate([W2, W2 @ a2s[0][:, None], W2 @ a2d[0][:, None]],
                            axis=1)  # [HD, 4]
    w2e = w2cols.reshape(2, P, 4).transpose(1, 0, 2).reshape(P, 8).astype(NP_BF)

    loops = np.arange(N, dtype=np.int64)
    src = np.concatenate([ei[0], loops])
    dst = np.concatenate([ei[1], loops])

    # Pass 1: per-core bucket assignment + counts. The gather's descriptor-ring
    # reservation (decode, from num_idxs_reg) must exactly match what the Q7
    # ucode pushes after trailing-negative trimming, and the program is shared
    # across cores, so the trim point is the max count over cores, rounded up
    # to the 128-idx reservation granularity. Slots [actual, reg) fetch row 0
    # (masked via dst=-1); slots [reg, NEH) are -1 and get trimmed.
    cnts = np.zeros((NCORES, NBLK * 2), dtype=np.int64)
    percore = []
    for c in range(NCORES):
        lo_n, hi_n = c * PER, (c + 1) * PER
        m = (dst >= lo_n) & (dst < hi_n)
        s_c = src[m]
        d_c = dst[m] - lo_n
        blk = d_c >> 7
        dloc = d_c & 127
        halfsel = (s_c >= HALF).astype(np.int64)
        key = blk * 2 + halfsel
        order = np.argsort(key, kind="stable")
        key_s = key[order]
        cnt = np.bincount(key_s, minlength=NBLK * 2)
        assert cnt.max() <= NEH, f"bucket overflow: {cnt.max()} > {NEH}"
        cnts[c] = cnt
        starts = np.zeros(NBLK * 2, dtype=np.int64)
        starts[1:] = np.cumsum(cnt)[:-1]
        pos = np.arange(len(key_s)) - starts[key_s]
        slot = key_s * NEH + pos
        sv = s_c[order] - halfsel[order] * HALF
        percore.append((slot, sv, dloc[order]))

    regs = np.minimum((cnts.max(axis=0) + 127) // 128 * 128, NEH)

    in_maps = []
    for c in range(NCORES):
        slot, sv, dl = percore[c]
        idxflat = np.zeros(NBLK * 2 * NEH, dtype=np.int16)
        dstflat = np.full(NBLK * 2 * NEH, -1.0, dtype=np.float32)
        idxflat[slot] = sv.astype(np.int16)
        dstflat[slot] = dl.astype(np.float32)

        idxw16 = (idxflat.reshape(NBLK * 2, NSUB * 8, 16)
                  .transpose(2, 0, 1).reshape(16, -1))
        idxw = np.tile(idxw16, (8, 1))  # replicated across the 8 Q7 cores
        dstw = (dstflat.reshape(NBLK * 2, NSUB, P).transpose(2, 0, 1)
                .reshape(P, NBLK * 2 * NSUB))
        dstb = np.ascontiguousarray(
            np.broadcast_to(dstflat.astype(NP_BF)[None, :], (P, NBLK * 2 * NEH)))

        in_maps.append({
            "xT": xT, "w1e": w1e, "w2e": w2e,
            "b1r": b1.reshape(1, HD).astype(np.float32),
            "b2r": b2.reshape(1, 2).astype(np.float32),
            "idx16": idxw, "dstf": np.ascontiguousarray(dstw),
            "dstb": dstb,
        })
    return in_maps, regs


_NC_CACHE = {}


def _get_nc(regs):
    key = tuple(int(r) for r in regs)
    if _NC_CACHE.get("key") != key:
        _NC_CACHE["nc"] = build_nc(FULL_CFG, regs)
        _NC_CACHE["key"] = key
    return _NC_CACHE["nc"]


def kernel(**inputs):
    from concourse.bass_utils import run_bass_kernel_spmd

    in_maps, regs = host_prep(inputs, FULL_CFG)
    nc = _get_nc(regs)
    res = run_bass_kernel_spmd(nc, in_maps, core_ids=list(range(FULL_CFG["NCORES"])))
    out = np.concatenate([r["out"] for r in res.results])[:FULL_CFG["N"]]
    return np.ascontiguousarray(out.astype(np.float32))
